# revision 22
# baseline (speedup 1.0000x reference)
"""Trainium2 Bass kernel for LogicGatedSNN.

Computes: spikes = (spike_input @ ternarize(synapse_states).T >= 1.0)
  where ternarize(s) = +1 if s > 1, -1 if s < -1, else 0.

Strategy:
  - 4x2 grid over 8 NeuronCores: batch split 4 ways (2048 rows/core),
    out_features split 2 ways (2048 cols/core). No collectives; the full
    output is assembled host-side from disjoint blocks. The 2-way W split
    halves per-core HBM traffic for W / ternarized-W versus pure data
    parallelism, and halves the DVE ternarize work.
  - Per core, BH=1024 batch rows per pass (2 passes), JS=256 output slabs:
    * X [2048, 4096] f32 is transposed on the TensorE (128x128 tiles via
      identity matmul into PSUM), then copied into k-major resident SBUF
      tiles (full f32 — no hi/lo split). 8 resident X^T tiles per pass.
    * W is ternarized to bf16 {-1,0,+1} (exact) with two DVE compares + add,
      staged to DRAM scratch per 512-row slab, reloaded k-major with the
      xbar transpose-DMA (2-byte dtype), then upcast bf16->f32 on the
      Scalar engine per k-tile just ahead of the matmuls. Ternarized W is
      re-read once per batch pass (2x total).
    * Matmul: single pass in float32r (PE "relaxed fp32": 1 col/cycle at
      moving free-dim >= 256, ~tf32-class precision — measured rms error
      3e-3 on K=4096 ternary accumulation, well inside the 2e-2 gate).
      Stationary = X^T tiles [128k, 128b], moving = W'^T [128k, 256j]
      (f32 tiles, APs bitcast to float32r), accumulated over 32 k-tiles.
      PSUM: two 256-col accumulators share each 2KB bank as ONE
      accumulation group (start on the even half's first matmul, stop on
      the odd half's last) so 8 batch tiles fit in 4 banks, leaving 4 for
      the X-transpose staging.
    * Spike threshold (>= 1.0) on DVE straight out of PSUM (one op per
      bank pair), emitted as bf16 0/1 (exact) to halve the output DMA;
      host casts back to f32.
  - vs the bf16 hi/lo double-pass baseline: half the PE matmul work.
"""

import sys

if "/opt/trn_rl_repo" not in sys.path:
    sys.path.insert(0, "/opt/trn_rl_repo")

import numpy as np

N_CORES = 8
BATCH, IN_F, OUT_F = 8192, 4096, 4096
GRID_B, GRID_J = 4, 2
B_CORE = BATCH // GRID_B  # 2048
J_CORE = OUT_F // GRID_J  # 2048

_BUILT = None


def build_bass(B, K, J, JS=256, KCH=1024, XCH=1024, reps=1, ring_split=False,
               w_first=True, BH=1024, xt_bufs=1, out_bf16=True, WBLK=256, TG=4,
               WFR=6):
    """Build the per-core Bass program for x:[B,K] f32, w:[J,K] f32 -> out:[B,J].

    reps > 1 repeats the whole compute (idempotent) for benchmarking via
    wall-clock deltas between builds with different reps.
    """
    from concourse import bacc
    import concourse.mybir as mybir
    import concourse.tile as tile

    f32, f32r, bf16 = mybir.dt.float32, mybir.dt.float32r, mybir.dt.bfloat16
    alu = mybir.AluOpType
    P = 128
    JS = min(JS, J)
    KCH = min(KCH, K)
    XCH = min(XCH, K)
    BH = min(BH, B)           # batch rows per pass
    NBP = B // BH             # number of batch passes
    BT = BH // P              # 128-row tiles per pass
    KT = K // P               # k tiles (partition-dim groups)
    NSLAB = J // JS           # output-feature slabs
    NKC = K // KCH            # W staging chunks along k
    NXC = K // XCH            # X staging chunks along k
    odt = bf16 if out_bf16 else f32
    assert B % BH == 0 and BH % P == 0 and K % P == 0 and J % JS == 0
    # Pair two accumulators per PSUM bank when a slab is a half-bank wide.
    pair = (JS * 4 * 2 <= 2048) and (BT % 2 == 0)

    nc = bacc.Bacc("TRN2", target_bir_lowering=False, debug=False)
    x = nc.dram_tensor("x", [B, K], f32, kind="ExternalInput")
    w = nc.dram_tensor("w", [J, K], f32, kind="ExternalInput")
    out = nc.dram_tensor("out", [B, J], odt, kind="ExternalOutput")

    with tile.TileContext(nc) as tc:
        with (
            tc.tile_pool(name="dram", bufs=1, space="DRAM") as dpool,
            tc.tile_pool(name="xstage32", bufs=3) as xs32,
            tc.tile_pool(name="wstage32", bufs=2) as ws32,
            tc.tile_pool(name="wstage16", bufs=2) as ws16,
            tc.tile_pool(name="xtres", bufs=xt_bufs) as xtres,
            tc.tile_pool(name="wtp", bufs=2) as wtp,
            tc.tile_pool(name="wfp", bufs=WFR) as wfp,
            tc.tile_pool(name="ostage", bufs=6) as op,
            tc.tile_pool(name="psum", bufs=1, space="PSUM") as pp,
        ):
            # DRAM scratch: ternarized W (natural layout). Separate tiles per
            # row-block keep RAW deps slab-granular for pipelining.
            WBLK = min(WBLK, J)
            wt_blocks = [
                dpool.tile([WBLK, K], bf16, name=f"wt_nat_r{r}")
                for r in range(J // WBLK)
            ]

            import itertools

            from concourse.masks import make_identity

            ident = xtres.tile([P, P], f32, name="ident")
            make_identity(nc, ident[:])

            xdma = nc.scalar if ring_split else nc.sync
            odma = nc.scalar

            def tern_rows(j0, js):
                # ternarize W rows [j0, j0+js) into wt_blocks
                for jsub in range(js // P):
                    jj = j0 + jsub * P
                    for kc in range(NKC):
                        c0 = kc * KCH
                        win = ws32.tile([P, KCH], f32, name="win")
                        nc.sync.dma_start(
                            out=win[:], in_=w[jj : jj + P, c0 : c0 + KCH]
                        )
                        a = ws16.tile([P, KCH], bf16, name="wpos")
                        nc.vector.tensor_scalar(
                            out=a[:], in0=win[:], scalar1=1.0, scalar2=None,
                            op0=alu.is_gt,
                        )
                        b2 = ws16.tile([P, KCH], bf16, name="wneg")
                        nc.vector.tensor_scalar(
                            out=b2[:], in0=win[:], scalar1=-1.0, scalar2=-1.0,
                            op0=alu.is_ge, op1=alu.add,
                        )
                        t = ws16.tile([P, KCH], bf16, name="wtern")
                        nc.vector.tensor_add(out=t[:], in0=a[:], in1=b2[:])
                        nc.sync.dma_start(
                            out=wt_blocks[jj // WBLK][
                                jj % WBLK : jj % WBLK + P, c0 : c0 + KCH
                            ],
                            in_=t[:],
                        )

            def wt_t_load(wt, j0, js):
                # transpose-load W'^T rows [j0, j0+js) into wt[:, :, 0:js]
                for i0 in range(0, js, WBLK):
                    blk = wt_blocks[(j0 + i0) // WBLK]
                    r = (j0 + i0) % WBLK
                    width = min(WBLK - r, js - i0)
                    nc.sync.dma_start_transpose(
                        out=wt[:, :, i0 : i0 + width],
                        in_=blk[r : r + width, :],
                    )

            # TG: k-tiles per PSUM transpose group (TG=4 -> one 2KB bank)
            acc_banks = (BT // 2) if pair else BT
            tp_bufs = max(1, min(4, 8 - acc_banks))
            # With paired accumulators and TG*P == 2*JS, the transpose staging
            # shares the accumulator rings (same 2KB-bank tiles), so the acc
            # rings get bufs=2 (all 8 banks) and slab boundaries never stall
            # on accumulator reuse.
            share_tp = pair and TG * P == 2 * JS
            acc_bufs = 2 if share_tp else 1
            ngrp = 0  # rotating index for shared tp allocations

            # ternarize granularity: do a slab's worth of rows just ahead of
            # first use, except the leading chunk which is done up front.
            tern_done = [False] * (J // WBLK)

            def tern_upto(j_end):
                for r in range((j_end + WBLK - 1) // WBLK):
                    if not tern_done[r]:
                        tern_rows(r * WBLK, WBLK)
                        tern_done[r] = True

            for rep, bp in itertools.product(range(reps), range(NBP)):
                if bp == 0:
                    # re-ternarize every rep so reps-delta benchmarking charges
                    # the full W pipeline to each rep
                    for r in range(len(tern_done)):
                        tern_done[r] = False
                    if w_first:
                        tern_upto(min(WBLK, J))
                # ---- X prep: PE-transpose 128x128 f32 tiles into PSUM, then
                # ACT-copy into the resident k-major f32 tiles.
                xtc = [
                    xtres.tile([P, KT, P], f32r, name=f"xtc{bsub}")
                    for bsub in range(BT)
                ]
                for bsub in range(BT):
                    r0 = bp * BH + bsub * P
                    xck = []
                    for cx in range(NXC):
                        xin = xs32.tile([P, XCH], f32, name="xin")
                        xdma.dma_start(
                            out=xin[:], in_=x[r0 : r0 + P, cx * XCH : (cx + 1) * XCH]
                        )
                        xck.append(xin)
                    for g in range(KT // TG):
                        if share_tp:
                            tp = pp.tile(
                                [P, TG, P], f32,
                                name=f"accp{ngrp % (BT // 2)}", bufs=acc_bufs,
                            )
                            ngrp += 1
                        else:
                            tp = pp.tile([P, TG, P], f32, name="tps", bufs=tp_bufs)
                        for i in range(TG):
                            kt = g * TG + i
                            xin = xck[(kt * P) // XCH]
                            o = (kt * P) % XCH
                            nc.tensor.transpose(
                                tp[:, i, :], xin[:, o : o + P], ident[:]
                            )
                        nc.scalar.copy(
                            out=xtc[bsub][:, g * TG : (g + 1) * TG, :], in_=tp[:]
                        )

                def emit_epilogue(banks, j0, js, bpp):
                    # threshold straight out of PSUM, store bf16 0/1
                    if pair:
                        for i in range(BT // 2):
                            spk = op.tile([P, 2 * JS], odt, name="spk")
                            nc.vector.tensor_scalar(
                                out=spk[:], in0=banks[i][:], scalar1=1.0,
                                scalar2=None, op0=alu.is_ge,
                            )
                            for h in range(2):
                                b = 2 * i + h
                                r0 = bpp * BH + b * P
                                odma.dma_start(
                                    out=out[r0 : r0 + P, j0 : j0 + js],
                                    in_=spk[:, h * JS : (h + 1) * JS],
                                )
                    else:
                        for b in range(BT):
                            spk = op.tile([P, JS], odt, name="spk")
                            nc.vector.tensor_scalar(
                                out=spk[:, 0:js], in0=banks[b][:, 0:js],
                                scalar1=1.0, scalar2=None, op0=alu.is_ge,
                            )
                            r0 = bpp * BH + b * P
                            odma.dma_start(
                                out=out[r0 : r0 + P, j0 : j0 + js],
                                in_=spk[:, 0:js],
                            )

                pending = None
                for s in range(NSLAB):
                    j0, js = s * JS, JS
                    if bp == 0:
                        tern_upto(min(j0 + 2 * JS if w_first else j0 + JS, J))

                    # ---- transpose-load W'^T slab: [128 kpart, KT, js] bf16,
                    # then upcast per k-tile to f32 on the Scalar engine.
                    wt = wtp.tile([P, KT, JS], bf16, name="wt")
                    wt_t_load(wt, j0, js)

                    # ---- matmuls (k outer, b inner) ----
                    if pair:
                        banks = [
                            pp.tile([P, 2 * JS], f32, name=f"accp{i}",
                                    bufs=acc_bufs)
                            for i in range(BT // 2)
                        ]

                        def acc_ap(b):
                            return banks[b // 2][:, (b % 2) * JS : (b % 2) * JS + js]
                    else:
                        banks = [
                            pp.tile([P, max(JS, 512)], f32, name=f"acc{b}", bufs=1)
                            for b in range(BT)
                        ]

                        def acc_ap(b):
                            return banks[b][:, 0:js]
                    for k in range(KT):
                        wf = wfp.tile([P, JS], f32r, name="wf")
                        nc.scalar.copy(out=wf[:, 0:js], in_=wt[:, k, 0:js])
                        wfr = wf[:, 0:js]
                        for b in range(BT):
                            if pair:
                                st = k == 0 and (b % 2 == 0)
                                sp = k == KT - 1 and (b % 2 == 1)
                            else:
                                st, sp = k == 0, k == KT - 1
                            nc.tensor.matmul(
                                acc_ap(b),
                                xtc[b][:, k, :],
                                wfr,
                                start=st,
                                stop=sp,
                                skip_group_check=pair,
                            )
                    # ---- deferred epilogue: emit the PREVIOUS slab's
                    # threshold/store now, so its out-DMA triggers never sit
                    # ahead of this slab's upcasts in the ACT queue.
                    if pending is not None:
                        emit_epilogue(*pending)
                    pending = (banks, j0, js, bp)
                # flush before the next batch pass's X-prep reuses the rings
                emit_epilogue(*pending)

    nc.compile()
    return nc


def _get_built():
    global _BUILT
    if _BUILT is None:
        _BUILT = build_bass(B_CORE, IN_F, J_CORE)
    return _BUILT


def make_in_maps(xs, ws):
    """Per-core input slices for the GRID_B x GRID_J layout."""
    maps = []
    for c in range(N_CORES):
        bi, ji = c // GRID_J, c % GRID_J
        maps.append(
            {
                "x": xs[bi * B_CORE : (bi + 1) * B_CORE],
                "w": ws[ji * J_CORE : (ji + 1) * J_CORE],
            }
        )
    return maps


def assemble(results):
    """Gather per-core output blocks into the full [BATCH, OUT_F] f32 array."""
    out = np.empty((BATCH, OUT_F), dtype=np.float32)
    for c in range(N_CORES):
        bi, ji = c // GRID_J, c % GRID_J
        out[
            bi * B_CORE : (bi + 1) * B_CORE, ji * J_CORE : (ji + 1) * J_CORE
        ] = np.asarray(results[c]["out"]).astype(np.float32)
    return out


def kernel(spike_input: np.ndarray, synapse_states: np.ndarray) -> np.ndarray:
    from concourse.bass_utils import run_bass_kernel_spmd

    nc = _get_built()
    xs = np.ascontiguousarray(spike_input, dtype=np.float32)
    ws = np.ascontiguousarray(synapse_states, dtype=np.float32)
    res = run_bass_kernel_spmd(
        nc, make_in_maps(xs, ws), core_ids=list(range(N_CORES))
    )
    return assemble(res.results)


# revision 23
# speedup vs baseline: 1.1366x; 1.1366x over previous
"""Trainium2 Bass kernel for LogicGatedSNN.

Computes: spikes = (spike_input @ ternarize(synapse_states).T >= 1.0)
  where ternarize(s) = +1 if s > 1, -1 if s < -1, else 0.

Strategy:
  - 4x2 grid over 8 NeuronCores: batch split 4 ways (2048 rows/core),
    out_features split 2 ways (2048 cols/core). No collectives; the full
    output is assembled host-side from disjoint blocks. The 2-way W split
    halves per-core HBM traffic for W / ternarized-W versus pure data
    parallelism, and halves the DVE ternarize work.
  - Per core, BH=1024 batch rows per pass (2 passes), JS=256 output slabs:
    * X [2048, 4096] f32 is transposed on the TensorE (128x128 tiles via
      identity matmul into PSUM), then copied into k-major resident SBUF
      tiles (full f32 — no hi/lo split). 8 resident X^T tiles per pass.
    * W is ternarized to bf16 {-1,0,+1} (exact) with two DVE compares + add,
      staged to DRAM scratch per 512-row slab, reloaded k-major with the
      xbar transpose-DMA (2-byte dtype), then upcast bf16->f32 on the
      Scalar engine per k-tile just ahead of the matmuls. Ternarized W is
      re-read once per batch pass (2x total).
    * Matmul: single pass in float32r (PE "relaxed fp32": 1 col/cycle at
      moving free-dim >= 256, ~tf32-class precision — measured rms error
      3e-3 on K=4096 ternary accumulation, well inside the 2e-2 gate).
      Stationary = X^T tiles [128k, 128b], moving = W'^T [128k, 256j]
      (f32 tiles, APs bitcast to float32r), accumulated over 32 k-tiles.
      PSUM: two 256-col accumulators share each 2KB bank as ONE
      accumulation group (start on the even half's first matmul, stop on
      the odd half's last) so 8 batch tiles fit in 4 banks, leaving 4 for
      the X-transpose staging.
    * Spike threshold (>= 1.0) on DVE straight out of PSUM (one op per
      bank pair), emitted as bf16 0/1 (exact) to halve the output DMA;
      host casts back to f32.
  - vs the bf16 hi/lo double-pass baseline: half the PE matmul work.
"""

import sys

if "/opt/trn_rl_repo" not in sys.path:
    sys.path.insert(0, "/opt/trn_rl_repo")

import numpy as np

N_CORES = 8
BATCH, IN_F, OUT_F = 8192, 4096, 4096
GRID_B, GRID_J = 4, 2
B_CORE = BATCH // GRID_B  # 2048
J_CORE = OUT_F // GRID_J  # 2048

_BUILT = None


def build_bass(B, K, J, JS=256, KCH=1024, XCH=1024, reps=1, ring_split=False,
               w_first=True, BH=1024, xt_bufs=1, out_bf16=True, WBLK=256, TG=4,
               WFR=6, share=True, defer=True):
    """Build the per-core Bass program for x:[B,K] f32, w:[J,K] f32 -> out:[B,J].

    reps > 1 repeats the whole compute (idempotent) for benchmarking via
    wall-clock deltas between builds with different reps.
    """
    from concourse import bacc
    import concourse.mybir as mybir
    import concourse.tile as tile

    f32, f32r, bf16 = mybir.dt.float32, mybir.dt.float32r, mybir.dt.bfloat16
    alu = mybir.AluOpType
    P = 128
    JS = min(JS, J)
    KCH = min(KCH, K)
    XCH = min(XCH, K)
    BH = min(BH, B)           # batch rows per pass
    NBP = B // BH             # number of batch passes
    BT = BH // P              # 128-row tiles per pass
    KT = K // P               # k tiles (partition-dim groups)
    NSLAB = J // JS           # output-feature slabs
    NKC = K // KCH            # W staging chunks along k
    NXC = K // XCH            # X staging chunks along k
    odt = bf16 if out_bf16 else f32
    assert B % BH == 0 and BH % P == 0 and K % P == 0 and J % JS == 0
    # Pair two accumulators per PSUM bank when a slab is a half-bank wide.
    pair = (JS * 4 * 2 <= 2048) and (BT % 2 == 0)

    nc = bacc.Bacc("TRN2", target_bir_lowering=False, debug=False)
    x = nc.dram_tensor("x", [B, K], f32, kind="ExternalInput")
    w = nc.dram_tensor("w", [J, K], f32, kind="ExternalInput")
    out = nc.dram_tensor("out", [B, J], odt, kind="ExternalOutput")

    with tile.TileContext(nc) as tc:
        with (
            tc.tile_pool(name="dram", bufs=1, space="DRAM") as dpool,
            tc.tile_pool(name="xstage32", bufs=3) as xs32,
            tc.tile_pool(name="wstage32", bufs=2) as ws32,
            tc.tile_pool(name="wstage16", bufs=2) as ws16,
            tc.tile_pool(name="xtres", bufs=xt_bufs) as xtres,
            tc.tile_pool(name="wtp", bufs=2) as wtp,
            tc.tile_pool(name="wfp", bufs=WFR) as wfp,
            tc.tile_pool(name="ostage", bufs=6) as op,
            tc.tile_pool(name="psum", bufs=1, space="PSUM") as pp,
        ):
            # DRAM scratch: ternarized W (natural layout). Separate tiles per
            # row-block keep RAW deps slab-granular for pipelining.
            WBLK = min(WBLK, J)
            wt_blocks = [
                dpool.tile([WBLK, K], bf16, name=f"wt_nat_r{r}")
                for r in range(J // WBLK)
            ]

            import itertools

            from concourse.masks import make_identity

            ident = xtres.tile([P, P], f32, name="ident")
            make_identity(nc, ident[:])

            xdma = nc.scalar if ring_split else nc.sync
            odma = nc.scalar

            def tern_rows(j0, js):
                # ternarize W rows [j0, j0+js) into wt_blocks
                for jsub in range(js // P):
                    jj = j0 + jsub * P
                    for kc in range(NKC):
                        c0 = kc * KCH
                        win = ws32.tile([P, KCH], f32, name="win")
                        nc.sync.dma_start(
                            out=win[:], in_=w[jj : jj + P, c0 : c0 + KCH]
                        )
                        a = ws16.tile([P, KCH], bf16, name="wpos")
                        nc.vector.tensor_scalar(
                            out=a[:], in0=win[:], scalar1=1.0, scalar2=None,
                            op0=alu.is_gt,
                        )
                        b2 = ws16.tile([P, KCH], bf16, name="wneg")
                        nc.vector.tensor_scalar(
                            out=b2[:], in0=win[:], scalar1=-1.0, scalar2=-1.0,
                            op0=alu.is_ge, op1=alu.add,
                        )
                        t = ws16.tile([P, KCH], bf16, name="wtern")
                        nc.vector.tensor_add(out=t[:], in0=a[:], in1=b2[:])
                        nc.sync.dma_start(
                            out=wt_blocks[jj // WBLK][
                                jj % WBLK : jj % WBLK + P, c0 : c0 + KCH
                            ],
                            in_=t[:],
                        )

            def wt_t_load(wt, j0, js):
                # transpose-load W'^T rows [j0, j0+js) into wt[:, :, 0:js]
                for i0 in range(0, js, WBLK):
                    blk = wt_blocks[(j0 + i0) // WBLK]
                    r = (j0 + i0) % WBLK
                    width = min(WBLK - r, js - i0)
                    nc.sync.dma_start_transpose(
                        out=wt[:, :, i0 : i0 + width],
                        in_=blk[r : r + width, :],
                    )

            # TG: k-tiles per PSUM transpose group (TG=4 -> one 2KB bank)
            acc_banks = (BT // 2) if pair else BT
            tp_bufs = max(1, min(4, 8 - acc_banks))
            # With paired accumulators and TG*P == 2*JS, the transpose staging
            # shares the accumulator rings (same 2KB-bank tiles), so the acc
            # rings get bufs=2 (all 8 banks) and slab boundaries never stall
            # on accumulator reuse.
            share_tp = share and pair and TG * P == 2 * JS
            acc_bufs = 2 if share_tp else 1
            ngrp = 0  # rotating index for shared tp allocations

            # ternarize granularity: do a slab's worth of rows just ahead of
            # first use, except the leading chunk which is done up front.
            tern_done = [False] * (J // WBLK)

            def tern_upto(j_end):
                for r in range((j_end + WBLK - 1) // WBLK):
                    if not tern_done[r]:
                        tern_rows(r * WBLK, WBLK)
                        tern_done[r] = True

            for rep, bp in itertools.product(range(reps), range(NBP)):
                if bp == 0:
                    # re-ternarize every rep so reps-delta benchmarking charges
                    # the full W pipeline to each rep
                    for r in range(len(tern_done)):
                        tern_done[r] = False
                    if w_first:
                        tern_upto(min(WBLK, J))
                # ---- X prep: PE-transpose 128x128 f32 tiles into PSUM, then
                # ACT-copy into the resident k-major f32 tiles.
                xtc = [
                    xtres.tile([P, KT, P], f32r, name=f"xtc{bsub}")
                    for bsub in range(BT)
                ]
                for bsub in range(BT):
                    r0 = bp * BH + bsub * P
                    xck = []
                    for cx in range(NXC):
                        xin = xs32.tile([P, XCH], f32, name="xin")
                        xdma.dma_start(
                            out=xin[:], in_=x[r0 : r0 + P, cx * XCH : (cx + 1) * XCH]
                        )
                        xck.append(xin)
                    for g in range(KT // TG):
                        if share_tp:
                            tp = pp.tile(
                                [P, TG, P], f32,
                                name=f"accp{ngrp % (BT // 2)}", bufs=acc_bufs,
                            )
                            ngrp += 1
                        else:
                            tp = pp.tile([P, TG, P], f32, name="tps", bufs=tp_bufs)
                        for i in range(TG):
                            kt = g * TG + i
                            xin = xck[(kt * P) // XCH]
                            o = (kt * P) % XCH
                            nc.tensor.transpose(
                                tp[:, i, :], xin[:, o : o + P], ident[:]
                            )
                        nc.scalar.copy(
                            out=xtc[bsub][:, g * TG : (g + 1) * TG, :], in_=tp[:]
                        )

                def emit_epilogue(banks, j0, js, bpp):
                    # threshold straight out of PSUM, store bf16 0/1
                    if pair:
                        for i in range(BT // 2):
                            spk = op.tile([P, 2 * JS], odt, name="spk")
                            nc.vector.tensor_scalar(
                                out=spk[:], in0=banks[i][:], scalar1=1.0,
                                scalar2=None, op0=alu.is_ge,
                            )
                            for h in range(2):
                                b = 2 * i + h
                                r0 = bpp * BH + b * P
                                odma.dma_start(
                                    out=out[r0 : r0 + P, j0 : j0 + js],
                                    in_=spk[:, h * JS : (h + 1) * JS],
                                )
                    else:
                        for b in range(BT):
                            spk = op.tile([P, JS], odt, name="spk")
                            nc.vector.tensor_scalar(
                                out=spk[:, 0:js], in0=banks[b][:, 0:js],
                                scalar1=1.0, scalar2=None, op0=alu.is_ge,
                            )
                            r0 = bpp * BH + b * P
                            odma.dma_start(
                                out=out[r0 : r0 + P, j0 : j0 + js],
                                in_=spk[:, 0:js],
                            )

                pending = None
                for s in range(NSLAB):
                    j0, js = s * JS, JS
                    if bp == 0:
                        tern_upto(min(j0 + 2 * JS if w_first else j0 + JS, J))

                    # ---- transpose-load W'^T slab: [128 kpart, KT, js] bf16,
                    # then upcast per k-tile to f32 on the Scalar engine.
                    wt = wtp.tile([P, KT, JS], bf16, name="wt")
                    wt_t_load(wt, j0, js)

                    # ---- matmuls (k outer, b inner) ----
                    if pair:
                        banks = [
                            pp.tile([P, 2 * JS], f32, name=f"accp{i}",
                                    bufs=acc_bufs)
                            for i in range(BT // 2)
                        ]

                        def acc_ap(b):
                            return banks[b // 2][:, (b % 2) * JS : (b % 2) * JS + js]
                    else:
                        banks = [
                            pp.tile([P, max(JS, 512)], f32, name=f"acc{b}", bufs=1)
                            for b in range(BT)
                        ]

                        def acc_ap(b):
                            return banks[b][:, 0:js]
                    for k in range(KT):
                        wf = wfp.tile([P, JS], f32r, name="wf")
                        nc.scalar.copy(out=wf[:, 0:js], in_=wt[:, k, 0:js])
                        wfr = wf[:, 0:js]
                        for b in range(BT):
                            if pair:
                                st = k == 0 and (b % 2 == 0)
                                sp = k == KT - 1 and (b % 2 == 1)
                            else:
                                st, sp = k == 0, k == KT - 1
                            nc.tensor.matmul(
                                acc_ap(b),
                                xtc[b][:, k, :],
                                wfr,
                                start=st,
                                stop=sp,
                                skip_group_check=pair,
                            )
                    # ---- deferred epilogue: emit the PREVIOUS slab's
                    # threshold/store now, so its out-DMA triggers never sit
                    # ahead of this slab's upcasts in the ACT queue.
                    if defer:
                        if pending is not None:
                            emit_epilogue(*pending)
                        pending = (banks, j0, js, bp)
                    else:
                        emit_epilogue(banks, j0, js, bp)
                # flush before the next batch pass's X-prep reuses the rings
                if defer:
                    emit_epilogue(*pending)

    nc.compile()
    return nc


def _get_built():
    global _BUILT
    if _BUILT is None:
        _BUILT = build_bass(B_CORE, IN_F, J_CORE)
    return _BUILT


def make_in_maps(xs, ws):
    """Per-core input slices for the GRID_B x GRID_J layout."""
    maps = []
    for c in range(N_CORES):
        bi, ji = c // GRID_J, c % GRID_J
        maps.append(
            {
                "x": xs[bi * B_CORE : (bi + 1) * B_CORE],
                "w": ws[ji * J_CORE : (ji + 1) * J_CORE],
            }
        )
    return maps


def assemble(results):
    """Gather per-core output blocks into the full [BATCH, OUT_F] f32 array."""
    out = np.empty((BATCH, OUT_F), dtype=np.float32)
    for c in range(N_CORES):
        bi, ji = c // GRID_J, c % GRID_J
        out[
            bi * B_CORE : (bi + 1) * B_CORE, ji * J_CORE : (ji + 1) * J_CORE
        ] = np.asarray(results[c]["out"]).astype(np.float32)
    return out


def kernel(spike_input: np.ndarray, synapse_states: np.ndarray) -> np.ndarray:
    from concourse.bass_utils import run_bass_kernel_spmd

    nc = _get_built()
    xs = np.ascontiguousarray(spike_input, dtype=np.float32)
    ws = np.ascontiguousarray(synapse_states, dtype=np.float32)
    res = run_bass_kernel_spmd(
        nc, make_in_maps(xs, ws), core_ids=list(range(N_CORES))
    )
    return assemble(res.results)
